# revision 1
# baseline (speedup 1.0000x reference)
"""Trainium2 Bass kernel for nn_CustomNetwork (4-layer 4096x4096 MLP with
train-mode BatchNorm1d + ReLU per layer, batch-axis softmax at the end).

Strategy: data-parallel over the batch dim across 8 NeuronCores (512 rows
per core). Activations live in SBUF transposed (channels on partitions,
batch on the free dim) so BatchNorm stats and the batch-axis softmax are
native free-axis reductions. Matmuls run in fp32r (full-rate fp32 on the
PE). Cross-core BatchNorm mean/var and the softmax exp-sum use AllReduce
over small per-channel vectors.

Note: the Linear bias `b` is mathematically canceled by BatchNorm's mean
subtraction, so it is never loaded.
"""

import numpy as np

import concourse.bacc as bacc
import concourse.mybir as mybir
import concourse.tile as tile
from concourse import bass_utils

P = 128  # SBUF partitions
D = 4096  # feature width
KT = D // P  # 32 k/n tiles
BM = 512  # per-core batch (4096 / 8 cores)
NSUP = 8  # n supertiles of 512 output channels
L = 4  # layers
N_CORES = 8
BN_EPS = 1e-5
# BN-stat allreduce chunking: first chunk issued early so its latency hides
# under the remaining matmuls; the small tail chunk is covered by the next
# layer's first k-steps.
CHUNKS = [(0, 24), (24, 32)]

F32 = mybir.dt.float32
F32R = mybir.dt.float32r

_cached_nc = None


def build():
    global _cached_nc
    if _cached_nc is not None:
        return _cached_nc
    nc = bacc.Bacc("TRN2", target_bir_lowering=False, debug=False, num_devices=N_CORES)

    xt = nc.dram_tensor("xt", [D, BM], F32, kind="ExternalInput")
    Wt = nc.dram_tensor("W", [L, D, D], F32, kind="ExternalInput")
    # gammaH/betaH are host-transposed to [L, P, KT] so the DMA runs with
    # 128B-contiguous lines (the natural [L, D] layout needs 4B descriptors
    # which cost ~5us per DMA and block the W prefetch queue)
    gamma = nc.dram_tensor("gammaH", [L, P, KT], F32, kind="ExternalInput")
    beta = nc.dram_tensor("betaH", [L, P, KT], F32, kind="ExternalInput")
    outt = nc.dram_tensor("outt", [D, BM], F32, kind="ExternalOutput")

    rg = [list(range(N_CORES))]

    with tile.TileContext(nc) as tc:
        with (
            tc.tile_pool(name="hbuf", bufs=1) as hpool,
            tc.tile_pool(name="wpool", bufs=16) as wpool,
            tc.tile_pool(name="psum", bufs=2, space="PSUM") as psum,
            tc.tile_pool(name="small", bufs=2) as small,
            tc.tile_pool(name="gb", bufs=1) as gbpool,
            tc.tile_pool(name="dram", bufs=1, space="DRAM") as dram,
        ):
            h = [
                hpool.tile([P, KT, BM], F32R, name="h_a"),
                hpool.tile([P, KT, BM], F32R, name="h_b"),
            ]

            # x^T -> h[0], interleaved with layer-0/ns-0 W prefetch so the
            # first matmuls start within a few us
            w_pre = []
            for k in range(KT):
                nc.sync.dma_start(
                    h[0][:, k, :], xt.ap()[k * P : (k + 1) * P, :].bitcast(F32R)
                )
                wt = wpool.tile([P, 512], F32R, name="wt")
                nc.sync.dma_start(
                    wt[:], Wt.ap()[0, k * P : (k + 1) * P, 0:512].bitcast(F32R)
                )
                w_pre.append(wt)

            # gamma/beta for all layers, laid out [p, tile] per layer
            gam = gbpool.tile([P, L, KT], F32, name="gam")
            bet = gbpool.tile([P, L, KT], F32, name="bet")
            for l in range(L):
                nc.sync.dma_start(gam[:, l, :], gamma.ap()[l])
                nc.sync.dma_start(bet[:, l, :], beta.ap()[l])

            sumexp = small.tile([P, KT], F32, name="sumexp")

            for l in range(L):
                src = h[l % 2]
                dst = h[(l + 1) % 2]
                dst32 = dst[:].bitcast(F32)

                stat6 = small.tile([P, KT, 6], F32, name=f"stat6_{l}")
                meanvar = small.tile([P, KT, 2], F32, name=f"meanvar_{l}")

                # ---- matmul phase: out^T[n, m] = sum_k W[k, n] * h^T[k, m]
                for ns in range(NSUP):
                    ps = psum.tile([P, 4, BM], F32, name="ps")
                    for k in range(KT):
                        if l == 0 and ns == 0:
                            wt = w_pre[k]
                        else:
                            wt = wpool.tile([P, 512], F32R, name="wt")
                            nc.sync.dma_start(
                                wt[:],
                                Wt.ap()[
                                    l, k * P : (k + 1) * P, ns * 512 : (ns + 1) * 512
                                ].bitcast(F32R),
                            )
                        for j in range(4):
                            nc.tensor.matmul(
                                ps[:, j, :],
                                wt[:, j * P : (j + 1) * P],
                                src[:, k, :],
                                start=(k == 0),
                                stop=(k == KT - 1),
                            )
                    for j in range(4):
                        t = ns * 4 + j
                        # pre-BN activations to SBUF (every writer of an h
                        # buffer must produce fp32r — walrus checks all
                        # writers of tensors consumed by fp32r matmuls)
                        nc.vector.tensor_copy(dst[:, t, :], ps[:, j, :])
                        # per-channel batch stats of this tile
                        nc.vector.bn_stats(stat6[:, t, :], ps[:, j, :])
                        nc.vector.bn_aggr(meanvar[:, t, :], stat6[:, t, :])

                # ---- BN: chunked cross-core mean / E[h^2] allreduce + apply
                for ci, (t0, t1) in enumerate(CHUNKS):
                    n = t1 - t0
                    pack = small.tile([P, 2, n], F32, name=f"pack_{l}_{ci}")
                    # pack[:,0,:] = local mean; pack[:,1,:] = var + mean^2
                    nc.vector.tensor_copy(pack[:, 0, :], meanvar[:, t0:t1, 0])
                    nc.vector.tensor_tensor(
                        pack[:, 1, :],
                        meanvar[:, t0:t1, 0],
                        meanvar[:, t0:t1, 0],
                        op=mybir.AluOpType.mult,
                    )
                    nc.vector.tensor_tensor(
                        pack[:, 1, :],
                        pack[:, 1, :],
                        meanvar[:, t0:t1, 1],
                        op=mybir.AluOpType.add,
                    )
                    ar_in = dram.tile([P, 2, n], F32, name=f"arin_{l}_{ci}")
                    ar_out = dram.tile([P, 2, n], F32, name=f"arout_{l}_{ci}")
                    nc.sync.dma_start(ar_in[:], pack[:])
                    nc.gpsimd.collective_compute(
                        "AllReduce",
                        mybir.AluOpType.add,
                        replica_groups=rg,
                        ins=[ar_in.opt()],
                        outs=[ar_out.opt()],
                    )
                    red = small.tile([P, 2, n], F32, name=f"red_{l}_{ci}")
                    nc.sync.dma_start(red[:], ar_out[:])

                    mean_g = small.tile([P, n], F32, name=f"mean_{l}_{ci}")
                    var_g = small.tile([P, n], F32, name=f"var_{l}_{ci}")
                    scale = small.tile([P, n], F32, name=f"scale_{l}_{ci}")
                    shift = small.tile([P, n], F32, name=f"shift_{l}_{ci}")
                    # global mean / E[h^2] (= sums / 8)
                    nc.vector.tensor_scalar_mul(mean_g[:], red[:, 0, :], 1.0 / N_CORES)
                    nc.vector.tensor_scalar_mul(var_g[:], red[:, 1, :], 1.0 / N_CORES)
                    # var = E[h^2] - mean^2
                    nc.vector.tensor_tensor(
                        scale[:], mean_g[:], mean_g[:], op=mybir.AluOpType.mult
                    )
                    nc.vector.tensor_sub(var_g[:], var_g[:], scale[:])
                    # scale = gamma / sqrt(var + eps); shift = beta - mean*scale
                    nc.vector.tensor_scalar_add(var_g[:], var_g[:], BN_EPS)
                    nc.scalar.activation(
                        scale[:],
                        var_g[:],
                        mybir.ActivationFunctionType.Sqrt,
                        bias=0.0,
                        scale=1.0,
                    )
                    nc.vector.reciprocal(scale[:], scale[:])
                    nc.vector.tensor_mul(scale[:], scale[:], gam[:, l, t0:t1])
                    nc.vector.tensor_tensor(
                        shift[:], mean_g[:], scale[:], op=mybir.AluOpType.mult
                    )
                    nc.vector.tensor_sub(shift[:], bet[:, l, t0:t1], shift[:])

                    # apply: h_next = relu(h_pre * scale + shift), in place,
                    # written as fp32r for the next layer's matmuls. On the
                    # final layer, fuse the softmax numerator instead:
                    # exp(relu(z)) = max(exp(z), 1); the DVE max also
                    # accumulates the per-channel sum for the denominator.
                    for i in range(n):
                        t = t0 + i
                        if l < L - 1:
                            nc.scalar.activation(
                                dst[:, t, :],
                                dst32[:, t, :],
                                mybir.ActivationFunctionType.Relu,
                                bias=shift[:, i : i + 1],
                                scale=scale[:, i : i + 1],
                            )
                        else:
                            nc.scalar.activation(
                                dst[:, t, :],
                                dst32[:, t, :],
                                mybir.ActivationFunctionType.Exp,
                                bias=shift[:, i : i + 1],
                                scale=scale[:, i : i + 1],
                            )
                            nc.vector.tensor_scalar(
                                dst[:, t, :],
                                dst32[:, t, :],
                                1.0,
                                0.0,
                                mybir.AluOpType.max,
                                mybir.AluOpType.add,
                                accum_out=sumexp[:, t : t + 1],
                            )

                    # final layer: softmax denominator allreduce + normalize +
                    # store for this chunk. Emitting it here (between the two
                    # BN-stat allreduces) lets chunk 0's sum-allreduce run on
                    # the TOPSP while the tail matmuls are still executing.
                    if l == L - 1:
                        ar_in2 = dram.tile([P, n], F32, name=f"sarin_{ci}")
                        ar_out2 = dram.tile([P, n], F32, name=f"sarout_{ci}")
                        nc.sync.dma_start(ar_in2[:], sumexp[:, t0:t1])
                        nc.gpsimd.collective_compute(
                            "AllReduce",
                            mybir.AluOpType.add,
                            replica_groups=rg,
                            ins=[ar_in2.opt()],
                            outs=[ar_out2.opt()],
                        )
                        rsum = small.tile([P, n], F32, name=f"rsum_{ci}")
                        nc.sync.dma_start(rsum[:], ar_out2[:])
                        nc.vector.reciprocal(rsum[:], rsum[:])
                        for i in range(n):
                            t = t0 + i
                            nc.vector.tensor_scalar_mul(
                                dst[:, t, :], dst32[:, t, :], rsum[:, i : i + 1]
                            )
                            nc.sync.dma_start(
                                outt.ap()[t * P : (t + 1) * P, :].bitcast(F32R),
                                dst[:, t, :],
                            )

    nc.compile()
    _cached_nc = nc
    return nc


def kernel(x, W, b, gamma, beta):
    """Full (unsharded) inputs -> full [4096, 4096] softmax output."""
    del b  # canceled by BatchNorm mean subtraction
    x = np.ascontiguousarray(x, dtype=np.float32)
    W = np.ascontiguousarray(W, dtype=np.float32)
    gamma = np.ascontiguousarray(gamma, dtype=np.float32)
    beta = np.ascontiguousarray(beta, dtype=np.float32)

    nc = build()
    # [L, D] -> [L, P, KT]: channel (t*128 + p) lands at [l, p, t], giving the
    # on-chip per-partition layout with contiguous DMA lines
    gammaH = np.ascontiguousarray(gamma.reshape(L, KT, P).transpose(0, 2, 1))
    betaH = np.ascontiguousarray(beta.reshape(L, KT, P).transpose(0, 2, 1))
    in_maps = []
    for c in range(N_CORES):
        xt_c = np.ascontiguousarray(x[c * BM : (c + 1) * BM, :].T)
        in_maps.append({"xt": xt_c, "W": W, "gammaH": gammaH, "betaH": betaH})
    r = bass_utils.run_bass_kernel_spmd(nc, in_maps, core_ids=list(range(N_CORES)))
    out = np.empty((N_CORES * BM, D), dtype=np.float32)
    for c in range(N_CORES):
        out[c * BM : (c + 1) * BM, :] = r.results[c]["outt"].T
    return out



# revision 9
# speedup vs baseline: 1.0245x; 1.0245x over previous
"""Trainium2 Bass kernel for nn_CustomNetwork (4-layer 4096x4096 MLP with
train-mode BatchNorm1d + ReLU per layer, batch-axis softmax at the end).

Strategy: data-parallel over the batch dim across 8 NeuronCores (512 rows
per core). Activations live in SBUF transposed (channels on partitions,
batch on the free dim) so BatchNorm stats and the batch-axis softmax are
native free-axis reductions. Matmul operands are bf16 (stationary weights
get the PE fast-weight-load path and half the HBM traffic; the PE cannot
mix 16/32-bit operands so activations are bf16 too). Pre-BN activations
are staged in fp32 (PSUM copies + BN stats) so the bf16 rounding is paid
only once per layer, on the post-BN write. Cross-core BatchNorm mean/var
and the softmax exp-sum use AllReduce over small per-channel vectors.

Note: the Linear bias `b` is mathematically canceled by BatchNorm's mean
subtraction, so it is never loaded.
"""

import ml_dtypes
import numpy as np

import concourse.bacc as bacc
import concourse.mybir as mybir
import concourse.tile as tile
from concourse import bass_utils

P = 128  # SBUF partitions
D = 4096  # feature width
KT = D // P  # 32 k/n tiles
BM = 512  # per-core batch (4096 / 8 cores)
NSUP = 8  # n supertiles of 512 output channels
L = 4  # layers
N_CORES = 8
BN_EPS = 1e-5
# BN-stat allreduce chunking: first chunk issued early so its latency hides
# under the remaining matmuls; the small tail chunk is covered by the next
# layer's first k-steps.
CHUNKS = [(0, 24), (24, 32)]

F32 = mybir.dt.float32
BF16 = mybir.dt.bfloat16

_cached_nc = None


def build():
    global _cached_nc
    if _cached_nc is not None:
        return _cached_nc
    nc = bacc.Bacc("TRN2", target_bir_lowering=False, debug=False, num_devices=N_CORES)

    xt = nc.dram_tensor("xt", [D, BM], BF16, kind="ExternalInput")
    Wt = nc.dram_tensor("W", [L, D, D], BF16, kind="ExternalInput")
    # gammaH/betaH are host-transposed to [L, P, KT] so the DMA runs with
    # 128B-contiguous lines (the natural [L, D] layout needs 4B descriptors
    # which cost ~5us per DMA and block the W prefetch queue)
    gamma = nc.dram_tensor("gammaH", [L, P, KT], F32, kind="ExternalInput")
    beta = nc.dram_tensor("betaH", [L, P, KT], F32, kind="ExternalInput")
    outt = nc.dram_tensor("outt", [D, BM], F32, kind="ExternalOutput")

    rg = [list(range(N_CORES))]

    with tile.TileContext(nc) as tc:
        with (
            tc.tile_pool(name="hbuf", bufs=1) as hpool,
            tc.tile_pool(name="wpool", bufs=24) as wpool,
            tc.tile_pool(name="psum", bufs=2, space="PSUM") as psum,
            tc.tile_pool(name="small", bufs=2) as small,
            tc.tile_pool(name="gb", bufs=1) as gbpool,
            tc.tile_pool(name="dram", bufs=1, space="DRAM") as dram,
        ):
            h = [
                hpool.tile([P, KT, BM], BF16, name="h_a"),
                hpool.tile([P, KT, BM], BF16, name="h_b"),
            ]
            # fp32 staging: pre-BN activations (PSUM drain target) and, on the
            # last layer, the normalized fp32 output before the store
            hpre = hpool.tile([P, KT, BM], F32, name="h_pre")

            # x^T -> h[0], interleaved with layer-0/ns-0 W prefetch so the
            # first matmuls start within a few us
            w_pre = []
            for k in range(KT):
                nc.sync.dma_start(h[0][:, k, :], xt.ap()[k * P : (k + 1) * P, :])
                wt = wpool.tile([P, 512], BF16, name="wt")
                nc.sync.dma_start(wt[:], Wt.ap()[0, k * P : (k + 1) * P, 0:512])
                w_pre.append(wt)

            # gamma/beta for all layers, laid out [p, tile] per layer
            gam = gbpool.tile([P, L, KT], F32, name="gam")
            bet = gbpool.tile([P, L, KT], F32, name="bet")
            for l in range(L):
                nc.sync.dma_start(gam[:, l, :], gamma.ap()[l])
                nc.sync.dma_start(bet[:, l, :], beta.ap()[l])

            sumexp = small.tile([P, KT], F32, name="sumexp")

            for l in range(L):
                src = h[l % 2]
                dst = h[(l + 1) % 2]

                stat6 = small.tile([P, KT, 6], F32, name=f"stat6_{l}")
                meanvar = small.tile([P, KT, 2], F32, name=f"meanvar_{l}")

                # ---- matmul phase: out^T[n, m] = sum_k W[k, n] * h^T[k, m]
                for ns in range(NSUP):
                    ps = psum.tile([P, 4, BM], F32, name="ps")
                    for k in range(KT):
                        if l == 0 and ns == 0:
                            wt = w_pre[k]
                        else:
                            wt = wpool.tile([P, 512], BF16, name="wt")
                            nc.sync.dma_start(
                                wt[:],
                                Wt.ap()[
                                    l, k * P : (k + 1) * P, ns * 512 : (ns + 1) * 512
                                ],
                            )
                        for j in range(4):
                            nc.tensor.matmul(
                                ps[:, j, :],
                                wt[:, j * P : (j + 1) * P],
                                src[:, k, :],
                                start=(k == 0),
                                stop=(k == KT - 1),
                            )
                    for j in range(4):
                        t = ns * 4 + j
                        # pre-BN activations to fp32 staging SBUF
                        nc.vector.tensor_copy(hpre[:, t, :], ps[:, j, :])
                        # per-channel batch stats of this tile
                        nc.vector.bn_stats(stat6[:, t, :], ps[:, j, :])
                        nc.vector.bn_aggr(meanvar[:, t, :], stat6[:, t, :])

                # ---- BN: chunked cross-core mean / E[h^2] allreduce + apply
                for ci, (t0, t1) in enumerate(CHUNKS):
                    n = t1 - t0
                    pack = small.tile([P, 2, n], F32, name=f"pack_{l}_{ci}")
                    # pack[:,0,:] = local mean; pack[:,1,:] = var + mean^2
                    nc.vector.tensor_copy(pack[:, 0, :], meanvar[:, t0:t1, 0])
                    nc.vector.tensor_tensor(
                        pack[:, 1, :],
                        meanvar[:, t0:t1, 0],
                        meanvar[:, t0:t1, 0],
                        op=mybir.AluOpType.mult,
                    )
                    nc.vector.tensor_tensor(
                        pack[:, 1, :],
                        pack[:, 1, :],
                        meanvar[:, t0:t1, 1],
                        op=mybir.AluOpType.add,
                    )
                    ar_in = dram.tile([P, 2, n], F32, name=f"arin_{l}_{ci}")
                    ar_out = dram.tile([P, 2, n], F32, name=f"arout_{l}_{ci}")
                    nc.sync.dma_start(ar_in[:], pack[:])
                    nc.gpsimd.collective_compute(
                        "AllReduce",
                        mybir.AluOpType.add,
                        replica_groups=rg,
                        ins=[ar_in.opt()],
                        outs=[ar_out.opt()],
                    )
                    red = small.tile([P, 2, n], F32, name=f"red_{l}_{ci}")
                    nc.sync.dma_start(red[:], ar_out[:])

                    mean_g = small.tile([P, n], F32, name=f"mean_{l}_{ci}")
                    var_g = small.tile([P, n], F32, name=f"var_{l}_{ci}")
                    scale = small.tile([P, n], F32, name=f"scale_{l}_{ci}")
                    shift = small.tile([P, n], F32, name=f"shift_{l}_{ci}")
                    # global mean / E[h^2] (= sums / 8)
                    nc.vector.tensor_scalar_mul(mean_g[:], red[:, 0, :], 1.0 / N_CORES)
                    nc.vector.tensor_scalar_mul(var_g[:], red[:, 1, :], 1.0 / N_CORES)
                    # var = E[h^2] - mean^2
                    nc.vector.tensor_tensor(
                        scale[:], mean_g[:], mean_g[:], op=mybir.AluOpType.mult
                    )
                    nc.vector.tensor_sub(var_g[:], var_g[:], scale[:])
                    # scale = gamma / sqrt(var + eps); shift = beta - mean*scale
                    nc.vector.tensor_scalar_add(var_g[:], var_g[:], BN_EPS)
                    nc.scalar.activation(
                        scale[:],
                        var_g[:],
                        mybir.ActivationFunctionType.Sqrt,
                        bias=0.0,
                        scale=1.0,
                    )
                    nc.vector.reciprocal(scale[:], scale[:])
                    nc.vector.tensor_mul(scale[:], scale[:], gam[:, l, t0:t1])
                    nc.vector.tensor_tensor(
                        shift[:], mean_g[:], scale[:], op=mybir.AluOpType.mult
                    )
                    nc.vector.tensor_sub(shift[:], bet[:, l, t0:t1], shift[:])

                    # apply: h_next = relu(h_pre * scale + shift), written as
                    # bf16 for the next layer's matmuls. On the final layer,
                    # fuse the softmax numerator instead:
                    # exp(relu(z)) = max(exp(z), 1); the DVE max also
                    # accumulates the per-channel sum for the denominator.
                    for i in range(n):
                        t = t0 + i
                        if l < L - 1:
                            nc.scalar.activation(
                                dst[:, t, :],
                                hpre[:, t, :],
                                mybir.ActivationFunctionType.Relu,
                                bias=shift[:, i : i + 1],
                                scale=scale[:, i : i + 1],
                            )
                        else:
                            nc.scalar.activation(
                                dst[:, t, :],
                                hpre[:, t, :],
                                mybir.ActivationFunctionType.Exp,
                                bias=shift[:, i : i + 1],
                                scale=scale[:, i : i + 1],
                            )
                            nc.vector.tensor_scalar(
                                dst[:, t, :],
                                dst[:, t, :],
                                1.0,
                                0.0,
                                mybir.AluOpType.max,
                                mybir.AluOpType.add,
                                accum_out=sumexp[:, t : t + 1],
                            )

                    # final layer: softmax denominator allreduce + normalize +
                    # store for this chunk. Emitting it here (between the two
                    # BN-stat allreduces) lets chunk 0's sum-allreduce run on
                    # the TOPSP while the tail matmuls are still executing.
                    if l == L - 1:
                        ar_in2 = dram.tile([P, n], F32, name=f"sarin_{ci}")
                        ar_out2 = dram.tile([P, n], F32, name=f"sarout_{ci}")
                        nc.sync.dma_start(ar_in2[:], sumexp[:, t0:t1])
                        nc.gpsimd.collective_compute(
                            "AllReduce",
                            mybir.AluOpType.add,
                            replica_groups=rg,
                            ins=[ar_in2.opt()],
                            outs=[ar_out2.opt()],
                        )
                        rsum = small.tile([P, n], F32, name=f"rsum_{ci}")
                        nc.sync.dma_start(rsum[:], ar_out2[:])
                        nc.vector.reciprocal(rsum[:], rsum[:])
                        for i in range(n):
                            t = t0 + i
                            # normalize into the fp32 staging tile, then store
                            nc.vector.tensor_scalar_mul(
                                hpre[:, t, :], dst[:, t, :], rsum[:, i : i + 1]
                            )
                            nc.sync.dma_start(
                                outt.ap()[t * P : (t + 1) * P, :], hpre[:, t, :]
                            )

    nc.compile()
    _cached_nc = nc
    return nc


def kernel(x, W, b, gamma, beta):
    """Full (unsharded) inputs -> full [4096, 4096] softmax output."""
    del b  # canceled by BatchNorm mean subtraction
    x = np.ascontiguousarray(x, dtype=np.float32)
    W = np.ascontiguousarray(np.asarray(W, dtype=np.float32).astype(ml_dtypes.bfloat16))
    gamma = np.ascontiguousarray(gamma, dtype=np.float32)
    beta = np.ascontiguousarray(beta, dtype=np.float32)

    nc = build()
    # [L, D] -> [L, P, KT]: channel (t*128 + p) lands at [l, p, t], giving the
    # on-chip per-partition layout with contiguous DMA lines
    gammaH = np.ascontiguousarray(gamma.reshape(L, KT, P).transpose(0, 2, 1))
    betaH = np.ascontiguousarray(beta.reshape(L, KT, P).transpose(0, 2, 1))
    in_maps = []
    for c in range(N_CORES):
        xt_c = np.ascontiguousarray(
            x[c * BM : (c + 1) * BM, :].T.astype(ml_dtypes.bfloat16)
        )
        in_maps.append({"xt": xt_c, "W": W, "gammaH": gammaH, "betaH": betaH})
    r = bass_utils.run_bass_kernel_spmd(nc, in_maps, core_ids=list(range(N_CORES)))
    out = np.empty((N_CORES * BM, D), dtype=np.float32)
    for c in range(N_CORES):
        out[c * BM : (c + 1) * BM, :] = r.results[c]["outt"].T
    return out


# revision 12
# speedup vs baseline: 1.3081x; 1.2768x over previous
"""Trainium2 Bass kernel for nn_CustomNetwork (4-layer 4096x4096 MLP with
train-mode BatchNorm1d + ReLU per layer, batch-axis softmax at the end).

Strategy: data-parallel over the batch dim across 8 NeuronCores (512 rows
per core). Activations live in SBUF transposed (channels on partitions,
batch on the free dim) so BatchNorm stats and the batch-axis softmax are
native free-axis reductions. Matmul operands are bf16 (stationary weights
get the PE fast-weight-load path and half the HBM traffic; the PE cannot
mix 16/32-bit operands so activations are bf16 too). Pre-BN activations
are staged in fp32 (PSUM copies + BN stats) so the bf16 rounding is paid
only once per layer, on the post-BN write. Cross-core BatchNorm mean/var
and the softmax exp-sum use AllReduce over small per-channel vectors.

BN scale = gamma*(var+eps)^-0.5 is computed with a DVE pow instead of the
ScalarE Sqrt: ScalarE then only ever runs Relu/Exp, avoiding the ~1.3us
activation-table reloads that otherwise land on the softmax tail.

Note: the Linear bias `b` is mathematically canceled by BatchNorm's mean
subtraction, so it is never loaded.
"""

import ml_dtypes
import numpy as np

import concourse.bacc as bacc
import concourse.mybir as mybir
import concourse.tile as tile
from concourse import bass_utils

P = 128  # SBUF partitions
D = 4096  # feature width
KT = D // P  # 32 k/n tiles
BM = 512  # per-core batch (4096 / 8 cores)
NSUP = 8  # n supertiles of 512 output channels
L = 4  # layers
N_CORES = 8
BN_EPS = 1e-5
# BN-stat allreduce chunking: first chunk issued early so its latency hides
# under the remaining matmuls; the small tail chunk is covered by the next
# layer's first k-steps.
CHUNKS = [(0, 24), (24, 32)]

F32 = mybir.dt.float32
BF16 = mybir.dt.bfloat16

_cached_nc = None


def build():
    global _cached_nc
    if _cached_nc is not None:
        return _cached_nc
    nc = bacc.Bacc("TRN2", target_bir_lowering=False, debug=False, num_devices=N_CORES)

    xt = nc.dram_tensor("xt", [D, BM], BF16, kind="ExternalInput")
    Wt = nc.dram_tensor("W", [L, D, D], BF16, kind="ExternalInput")
    # gammaH/betaH are host-transposed to [L, P, KT] so the DMA runs with
    # 128B-contiguous lines (the natural [L, D] layout needs 4B descriptors
    # which cost ~5us per DMA and block the W prefetch queue)
    gamma = nc.dram_tensor("gammaH", [L, P, KT], F32, kind="ExternalInput")
    beta = nc.dram_tensor("betaH", [L, P, KT], F32, kind="ExternalInput")
    outt = nc.dram_tensor("outt", [D, BM], F32, kind="ExternalOutput")

    rg = [list(range(N_CORES))]

    with tile.TileContext(nc) as tc:
        with (
            tc.tile_pool(name="hbuf", bufs=1) as hpool,
            tc.tile_pool(name="wpool", bufs=24) as wpool,
            tc.tile_pool(name="psum", bufs=2, space="PSUM") as psum,
            tc.tile_pool(name="small", bufs=2) as small,
            tc.tile_pool(name="gb", bufs=1) as gbpool,
            tc.tile_pool(name="dram", bufs=1, space="DRAM") as dram,
        ):
            h = [
                hpool.tile([P, KT, BM], BF16, name="h_a"),
                hpool.tile([P, KT, BM], BF16, name="h_b"),
            ]
            # fp32 staging: pre-BN activations (PSUM drain target) and, on the
            # last layer, the normalized fp32 output before the store
            hpre = hpool.tile([P, KT, BM], F32, name="h_pre")

            # x^T -> h[0], interleaved with layer-0/ns-0 W prefetch so the
            # first matmuls start within a few us
            w_pre = []
            for k in range(KT):
                nc.sync.dma_start(h[0][:, k, :], xt.ap()[k * P : (k + 1) * P, :])
                wt = wpool.tile([P, 512], BF16, name="wt")
                nc.sync.dma_start(wt[:], Wt.ap()[0, k * P : (k + 1) * P, 0:512])
                w_pre.append(wt)

            # gamma/beta for all layers, laid out [p, tile] per layer
            gam = gbpool.tile([P, L, KT], F32, name="gam")
            bet = gbpool.tile([P, L, KT], F32, name="bet")
            for l in range(L):
                nc.sync.dma_start(gam[:, l, :], gamma.ap()[l])
                nc.sync.dma_start(bet[:, l, :], beta.ap()[l])

            sumexp = small.tile([P, KT], F32, name="sumexp")

            def bn_pack_and_allreduce(l, ci, meanvar):
                """Pack local mean / E[h^2] for chunk ci and fire the AR."""
                t0, t1 = CHUNKS[ci]
                n = t1 - t0
                pack = small.tile([P, 2, n], F32, name=f"pack_{l}_{ci}")
                nc.vector.tensor_copy(pack[:, 0, :], meanvar[:, t0:t1, 0])
                nc.vector.tensor_tensor(
                    pack[:, 1, :],
                    meanvar[:, t0:t1, 0],
                    meanvar[:, t0:t1, 0],
                    op=mybir.AluOpType.mult,
                )
                nc.vector.tensor_tensor(
                    pack[:, 1, :],
                    pack[:, 1, :],
                    meanvar[:, t0:t1, 1],
                    op=mybir.AluOpType.add,
                )
                ar_in = dram.tile([P, 2, n], F32, name=f"arin_{l}_{ci}")
                ar_out = dram.tile([P, 2, n], F32, name=f"arout_{l}_{ci}")
                nc.sync.dma_start(ar_in[:], pack[:])
                nc.gpsimd.collective_compute(
                    "AllReduce",
                    mybir.AluOpType.add,
                    replica_groups=rg,
                    ins=[ar_in.opt()],
                    outs=[ar_out.opt()],
                )
                red = small.tile([P, 2, n], F32, name=f"red_{l}_{ci}")
                nc.sync.dma_start(red[:], ar_out[:])
                return red

            def bn_scale_shift(l, ci, red):
                """Global mean/var from the AR sums -> (scale, shift)."""
                t0, t1 = CHUNKS[ci]
                n = t1 - t0
                mean_g = small.tile([P, n], F32, name=f"mean_{l}_{ci}")
                var_g = small.tile([P, n], F32, name=f"var_{l}_{ci}")
                scale = small.tile([P, n], F32, name=f"scale_{l}_{ci}")
                shift = small.tile([P, n], F32, name=f"shift_{l}_{ci}")
                nc.vector.tensor_scalar_mul(mean_g[:], red[:, 0, :], 1.0 / N_CORES)
                nc.vector.tensor_scalar_mul(var_g[:], red[:, 1, :], 1.0 / N_CORES)
                # var = E[h^2] - mean^2
                nc.vector.tensor_tensor(
                    scale[:], mean_g[:], mean_g[:], op=mybir.AluOpType.mult
                )
                nc.vector.tensor_sub(var_g[:], var_g[:], scale[:])
                nc.vector.tensor_scalar_add(var_g[:], var_g[:], BN_EPS)
                # scale = gamma * exp(-0.5*ln(var+eps)). Ln/Exp/Relu share one
                # ScalarE activation table (natural_log_exp_and_others), so
                # unlike Sqrt this costs no ~1.3us table reloads on the
                # BN->softmax critical path.
                nc.scalar.activation(
                    var_g[:],
                    var_g[:],
                    mybir.ActivationFunctionType.Ln,
                    bias=0.0,
                    scale=1.0,
                )
                nc.scalar.activation(
                    scale[:],
                    var_g[:],
                    mybir.ActivationFunctionType.Exp,
                    bias=0.0,
                    scale=-0.5,
                )
                nc.vector.tensor_mul(scale[:], scale[:], gam[:, l, t0:t1])
                nc.vector.tensor_tensor(
                    shift[:], mean_g[:], scale[:], op=mybir.AluOpType.mult
                )
                nc.vector.tensor_sub(shift[:], bet[:, l, t0:t1], shift[:])
                return scale, shift

            def bn_apply(l, ci, dst, scale, shift):
                """h_next = relu(h_pre*scale+shift) (hidden layers) or the
                softmax numerator max(exp(h_pre*scale+shift), 1) with the
                per-channel sum accumulated (last layer)."""
                t0, t1 = CHUNKS[ci]
                for i in range(t1 - t0):
                    t = t0 + i
                    if l < L - 1:
                        nc.scalar.activation(
                            dst[:, t, :],
                            hpre[:, t, :],
                            mybir.ActivationFunctionType.Relu,
                            bias=shift[:, i : i + 1],
                            scale=scale[:, i : i + 1],
                        )
                    else:
                        nc.scalar.activation(
                            dst[:, t, :],
                            hpre[:, t, :],
                            mybir.ActivationFunctionType.Exp,
                            bias=shift[:, i : i + 1],
                            scale=scale[:, i : i + 1],
                        )
                        nc.vector.tensor_scalar(
                            dst[:, t, :],
                            dst[:, t, :],
                            1.0,
                            0.0,
                            mybir.AluOpType.max,
                            mybir.AluOpType.add,
                            accum_out=sumexp[:, t : t + 1],
                        )

            def sumexp_allreduce(ci):
                t0, t1 = CHUNKS[ci]
                n = t1 - t0
                ar_in2 = dram.tile([P, n], F32, name=f"sarin_{ci}")
                ar_out2 = dram.tile([P, n], F32, name=f"sarout_{ci}")
                nc.sync.dma_start(ar_in2[:], sumexp[:, t0:t1])
                nc.gpsimd.collective_compute(
                    "AllReduce",
                    mybir.AluOpType.add,
                    replica_groups=rg,
                    ins=[ar_in2.opt()],
                    outs=[ar_out2.opt()],
                )
                rsum = small.tile([P, n], F32, name=f"rsum_{ci}")
                nc.sync.dma_start(rsum[:], ar_out2[:])
                return rsum

            def normalize_store(ci, dst, rsum):
                """out = exp/sumexp via fp32 staging, then store the chunk."""
                t0, t1 = CHUNKS[ci]
                nc.vector.reciprocal(rsum[:], rsum[:])
                for i in range(t1 - t0):
                    t = t0 + i
                    nc.vector.tensor_scalar_mul(
                        hpre[:, t, :], dst[:, t, :], rsum[:, i : i + 1]
                    )
                    nc.sync.dma_start(
                        outt.ap()[t * P : (t + 1) * P, :], hpre[:, t, :]
                    )

            for l in range(L):
                src = h[l % 2]
                dst = h[(l + 1) % 2]

                stat6 = small.tile([P, KT, 6], F32, name=f"stat6_{l}")
                meanvar = small.tile([P, KT, 2], F32, name=f"meanvar_{l}")

                # ---- matmul phase: out^T[n, m] = sum_k W[k, n] * h^T[k, m]
                for ns in range(NSUP):
                    ps = psum.tile([P, 4, BM], F32, name="ps")
                    for k in range(KT):
                        if l == 0 and ns == 0:
                            wt = w_pre[k]
                        else:
                            wt = wpool.tile([P, 512], BF16, name="wt")
                            nc.sync.dma_start(
                                wt[:],
                                Wt.ap()[
                                    l, k * P : (k + 1) * P, ns * 512 : (ns + 1) * 512
                                ],
                            )
                        for j in range(4):
                            nc.tensor.matmul(
                                ps[:, j, :],
                                wt[:, j * P : (j + 1) * P],
                                src[:, k, :],
                                start=(k == 0),
                                stop=(k == KT - 1),
                            )
                    for j in range(4):
                        t = ns * 4 + j
                        # pre-BN activations to fp32 staging SBUF. Only this
                        # single DVE pass touches PSUM: stats then read the
                        # SBUF copy, halving PSUM-port contention with the
                        # PE's accumulating writes.
                        nc.vector.tensor_copy(hpre[:, t, :], ps[:, j, :])
                        nc.vector.bn_stats(stat6[:, t, :], hpre[:, t, :])
                        nc.vector.bn_aggr(meanvar[:, t, :], stat6[:, t, :])

                if l < L - 1:
                    # ---- BN: chunked cross-core stat allreduce + apply. The
                    # tail chunk's AR latency hides under the next layer's
                    # first 24 k-steps (its tiles are consumed last there).
                    for ci in range(len(CHUNKS)):
                        red = bn_pack_and_allreduce(l, ci, meanvar)
                        scale, shift = bn_scale_shift(l, ci, red)
                        bn_apply(l, ci, dst, scale, shift)
                else:
                    # ---- last layer: interleave the two BN chunks with the
                    # two softmax-denominator ARs so the CC queue stays busy
                    # in dependency order and the vector queue never blocks
                    # chunk 1's BN math behind chunk 0's normalizes.
                    red0 = bn_pack_and_allreduce(l, 0, meanvar)
                    scale0, shift0 = bn_scale_shift(l, 0, red0)
                    bn_apply(l, 0, dst, scale0, shift0)
                    rsum0 = sumexp_allreduce(0)  # rides during supertile 7
                    red1 = bn_pack_and_allreduce(l, 1, meanvar)  # fires at end
                    normalize_store(0, dst, rsum0)
                    scale1, shift1 = bn_scale_shift(l, 1, red1)
                    bn_apply(l, 1, dst, scale1, shift1)
                    rsum1 = sumexp_allreduce(1)
                    normalize_store(1, dst, rsum1)

    nc.compile()
    _cached_nc = nc
    return nc


def kernel(x, W, b, gamma, beta):
    """Full (unsharded) inputs -> full [4096, 4096] softmax output."""
    del b  # canceled by BatchNorm mean subtraction
    x = np.ascontiguousarray(x, dtype=np.float32)
    W = np.ascontiguousarray(np.asarray(W, dtype=np.float32).astype(ml_dtypes.bfloat16))
    gamma = np.ascontiguousarray(gamma, dtype=np.float32)
    beta = np.ascontiguousarray(beta, dtype=np.float32)

    nc = build()
    # [L, D] -> [L, P, KT]: channel (t*128 + p) lands at [l, p, t], giving the
    # on-chip per-partition layout with contiguous DMA lines
    gammaH = np.ascontiguousarray(gamma.reshape(L, KT, P).transpose(0, 2, 1))
    betaH = np.ascontiguousarray(beta.reshape(L, KT, P).transpose(0, 2, 1))
    in_maps = []
    for c in range(N_CORES):
        xt_c = np.ascontiguousarray(
            x[c * BM : (c + 1) * BM, :].T.astype(ml_dtypes.bfloat16)
        )
        in_maps.append({"xt": xt_c, "W": W, "gammaH": gammaH, "betaH": betaH})
    r = bass_utils.run_bass_kernel_spmd(nc, in_maps, core_ids=list(range(N_CORES)))
    out = np.empty((N_CORES * BM, D), dtype=np.float32)
    for c in range(N_CORES):
        out[c * BM : (c + 1) * BM, :] = r.results[c]["outt"].T
    return out
